# revision 10
# baseline (speedup 1.0000x reference)
"""Trainium2 Bass kernel for nn_BeatGenerator: 4-layer "bidirectional" GRU
(8 cells scanning forward) + small input/output MLPs, batch 2048.

Strategy: pure data parallel over 8 NeuronCores (256 samples/core).
Per-core layout keeps batch on the SBUF free dim and features/gates on
partitions, so all GRU weights stay as matmul stationary operands
(lhsT [K<=128, gate 128]) and every matmul streams N=256 per direction.
Matmuls run as float32r (full PE rate, ~tf32 accuracy).  The pose MLP is
folded into the layer-0 GRU input weights on the host (LeakyReLU(1.0) is
identity, so it is one linear map).  Gate nonlinearities run on ScalarE,
the GRU blend on VectorE; the `inn + r*hn` add is done on the PE via an
identity-matmul PSUM accumulate.
"""

import numpy as np

BS, T, SIDE, H, Z, NSPK = 2048, 34, 30, 128, 16, 1024
NCORES = 8
B = BS // NCORES  # batch per core

_CACHE = {}


# --------------------------------------------------------------------------
# program builder
# --------------------------------------------------------------------------

def _build(has_bias: bool, t_steps: int = T, b: int = B, num_devices: int = NCORES):
    import concourse.bacc as bacc
    import concourse.mybir as mybir
    import concourse.tile as tile

    f32 = mybir.dt.float32
    f32r = mybir.dt.float32r
    AF = mybir.ActivationFunctionType
    OP = mybir.AluOpType

    nc = bacc.Bacc(
        "TRN2", target_bir_lowering=False, debug=False, enable_asserts=True,
        num_devices=num_devices,
    )

    # ---- DRAM I/O -------------------------------------------------------
    PAD = t_steps * SIDE + 2 * SIDE
    beats_t = nc.dram_tensor("beats_t", (2, PAD, b), f32r, kind="ExternalInput").ap()
    pre_t = nc.dram_tensor("pre_t", (t_steps, 28, b), f32r, kind="ExternalInput").ap()
    spk_t = nc.dram_tensor("spk_t", (Z, b), f32r, kind="ExternalInput").ap()
    eps_t = nc.dram_tensor("eps_t", (Z, b), f32, kind="ExternalInput").ap()

    wnames = {}

    def wdram(name, shape, dt_=None):
        wnames[name] = shape
        return nc.dram_tensor(name, shape, dt_ or f32r, kind="ExternalInput").ap()

    # GRU weights as lhsT tiles [K, 384]
    wx, wh = {}, {}
    for d in range(2):
        wx[(0, d)] = [wdram(f"wxs0_{d}", (128, 384)), wdram(f"wxp0_{d}", (28, 384))]
        wh[(0, d)] = wdram(f"whh0_{d}", (128, 384))
    for l in range(1, 4):
        for d in range(2):
            wx[(l, d)] = [wdram(f"wxa{l}_{d}", (128, 384)),
                          wdram(f"wxb{l}_{d}", (128, 384))]
            wh[(l, d)] = wdram(f"whh{l}_{d}", (128, 384))
    woh = wdram("woh", (128, 128))
    woz = wdram("woz", (16, 128))
    wb1 = wdram("wb1", (128, 64))
    wb2 = wdram("wb2", (64, 27))
    wsqu = wdram("wsqu", (90, 128))
    woen = wdram("woen", (90, 128))
    wspk = wdram("wspk", (16, 16))
    wmu = wdram("wmu", (16, 16))
    wlv = wdram("wlv", (16, 16))
    ident_d = wdram("ident", (128, 128))

    bias_d = {}
    if has_bias:
        # per (l, d): r, z, in, hn bias columns; plus head/input/z biases
        for l in range(4):
            for d in range(2):
                for g in ("r", "z", "in", "hn"):
                    bias_d[f"b{g}_{l}_{d}"] = wdram(f"b{g}_{l}_{d}", (128, 1), f32)
        bias_d["b_out"] = wdram("b_out", (128, 1), f32)
        bias_d["b_b1"] = wdram("b_b1", (64, 1), f32)
        bias_d["b_b2"] = wdram("b_b2", (27, 1), f32)
        bias_d["b_sum"] = wdram("b_sum", (128, 1), f32)
        bias_d["b_spk"] = wdram("b_spk", (16, 1), f32)
        bias_d["b_mu"] = wdram("b_mu", (16, 1), f32)
        bias_d["b_lv"] = wdram("b_lv", (16, 1), f32)      # full logvar bias
        bias_d["b_lvh"] = wdram("b_lvh", (16, 1), f32)    # 0.5 * logvar bias

    beat_o = nc.dram_tensor("beat_o", (t_steps, 27, b), f32, kind="ExternalOutput").ap()
    zc_o = nc.dram_tensor("zc_o", (Z, b), f32, kind="ExternalOutput").ap()
    zmu_o = nc.dram_tensor("zmu_o", (Z, b), f32, kind="ExternalOutput").ap()
    zlv_o = nc.dram_tensor("zlv_o", (Z, b), f32, kind="ExternalOutput").ap()

    def r_(ap):
        return ap if ap.dtype == f32r else ap.bitcast(f32r)

    with tile.TileContext(nc) as tc:
        from contextlib import ExitStack
        ctx = ExitStack()
        with ctx:
            wp = ctx.enter_context(tc.tile_pool(name="w", bufs=1))
            sp = ctx.enter_context(tc.tile_pool(name="s", bufs=2))
            hp = ctx.enter_context(tc.tile_pool(name="h", bufs=2))
            ps = ctx.enter_context(tc.tile_pool(name="ps", bufs=2, space="PSUM"))

            # ---- load weights into SBUF --------------------------------
            def wload(ap_dram, name):
                t_ = wp.tile(list(ap_dram.shape), ap_dram.dtype, name=name, tag=name)
                nc.sync.dma_start(out=t_, in_=ap_dram)
                return t_

            wx_s, wh_s = {}, {}
            for k, v in wx.items():
                wx_s[k] = [wload(a, f"wx{k[0]}{k[1]}{i}") for i, a in enumerate(v)]
            for k, v in wh.items():
                wh_s[k] = wload(v, f"wh{k[0]}{k[1]}")
            woh_s = wload(woh, "wohs")
            woz_s = wload(woz, "wozs")
            wb1_s = wload(wb1, "wb1s")
            wb2_s = wload(wb2, "wb2s")
            wsqu_s = wload(wsqu, "wsqus")
            woen_s = wload(woen, "woens")
            wspk_s = wload(wspk, "wspks")
            wmu_s = wload(wmu, "wmus")
            wlv_s = wload(wlv, "wlvs")
            ident = wload(ident_d, "idents")
            bias_s = {k: wload(v, k + "s") for k, v in bias_d.items()}

            def bget(key):
                return bias_s[key] if has_bias else 0.0

            # ---- speaker latent path -----------------------------------
            spk_sb = sp.tile([Z, b], f32r, tag="zsmall_r", bufs=4)
            nc.sync.dma_start(out=spk_sb, in_=spk_t)
            eps_sb = sp.tile([Z, b], f32, tag="zsmall", bufs=8)
            nc.sync.dma_start(out=eps_sb, in_=eps_t)

            zc0_p = ps.tile([Z, b], f32, tag="sm")
            nc.tensor.matmul(out=zc0_p, lhsT=r_(wspk_s), rhs=r_(spk_sb),
                             start=True, stop=True)
            zc0_sb = sp.tile([Z, b], f32r, tag="zsmall_r", bufs=4)
            if has_bias:
                nc.scalar.activation(zc0_sb, zc0_p, AF.Identity, bias=bget("b_spk"))
            else:
                nc.scalar.copy(zc0_sb, zc0_p)

            zmu_p = ps.tile([Z, b], f32, tag="sm")
            nc.tensor.matmul(out=zmu_p, lhsT=r_(wmu_s), rhs=r_(zc0_sb),
                             start=True, stop=True)
            zmu_sb = sp.tile([Z, b], f32, tag="zsmall", bufs=8)
            if has_bias:
                nc.scalar.activation(zmu_sb, zmu_p, AF.Identity, bias=bget("b_mu"))
            else:
                nc.scalar.copy(zmu_sb, zmu_p)
            nc.sync.dma_start(out=zmu_o, in_=zmu_sb)

            zlv_p = ps.tile([Z, b], f32, tag="sm")
            nc.tensor.matmul(out=zlv_p, lhsT=r_(wlv_s), rhs=r_(zc0_sb),
                             start=True, stop=True)
            zlv_sb = sp.tile([Z, b], f32, tag="zsmall", bufs=8)
            if has_bias:
                nc.scalar.activation(zlv_sb, zlv_p, AF.Identity, bias=bget("b_lv"))
            else:
                nc.scalar.copy(zlv_sb, zlv_p)
            nc.sync.dma_start(out=zlv_o, in_=zlv_sb)

            ee_sb = sp.tile([Z, b], f32, tag="zsmall", bufs=8)
            nc.scalar.activation(ee_sb, zlv_p, AF.Exp, scale=0.5,
                                 bias=bget("b_lvh") if has_bias else 0.0)
            ep_sb = sp.tile([Z, b], f32, tag="zsmall", bufs=8)
            nc.vector.tensor_tensor(ep_sb, ee_sb, eps_sb, op=OP.mult)
            zctx_sb = sp.tile([Z, b], f32r, tag="zctx", bufs=1)
            nc.vector.tensor_tensor(zctx_sb, zmu_sb, ep_sb, op=OP.add)
            nc.sync.dma_start(out=zc_o, in_=zctx_sb.bitcast(f32))

            # ---- GRU scan ----------------------------------------------
            h_cur = [None, None, None, None]

            for t in range(t_steps):
                # input MLP: sum_out(t) = win0 @ squ_w.T + win1 @ oenv_w.T
                win0 = sp.tile([90, b], f32r, tag="win0", bufs=3)
                nc.sync.dma_start(out=win0, in_=beats_t[0, SIDE * t:SIDE * t + 90, :])
                win1 = sp.tile([90, b], f32r, tag="win1", bufs=3)
                nc.sync.dma_start(out=win1, in_=beats_t[1, SIDE * t:SIDE * t + 90, :])
                sum_p = ps.tile([128, b], f32, tag="sm")
                nc.tensor.matmul(out=sum_p, lhsT=r_(wsqu_s), rhs=r_(win0),
                                 start=True, stop=False)
                nc.tensor.matmul(out=sum_p, lhsT=r_(woen_s), rhs=r_(win1),
                                 start=False, stop=True)
                sum_sb = sp.tile([128, b], f32r, tag="sum", bufs=3)
                if has_bias:
                    nc.scalar.activation(sum_sb, sum_p, AF.Identity,
                                         bias=bget("b_sum"))
                else:
                    nc.scalar.copy(sum_sb, sum_p)
                pre_sb = sp.tile([28, b], f32r, tag="pre", bufs=3)
                nc.sync.dma_start(out=pre_sb, in_=pre_t[t])

                for l in range(4):
                    if l == 0:
                        xkt = [(sum_sb, 0), (pre_sb, 1)]
                    else:
                        xprev = h_cur[l - 1]
                        xkt = [(xprev[:, 0:b], 0), (xprev[:, b:2 * b], 1)]

                    R = ps.tile([128, 2 * b], f32, tag="r")
                    Zp = ps.tile([128, 2 * b], f32, tag="z", bufs=1)
                    INN = ps.tile([128, 2 * b], f32, tag="inn")
                    HN = (ps.tile([128, 2 * b], f32, tag="hn", bufs=1, name="HN")
                          if t > 0 else None)

                    # Build per-bank ordered MM lists; exactly one start=True
                    # (first MM, lazily zeroing the whole bank) and one
                    # stop=True (last MM) per PSUM bank.
                    for gi_, (P, glo) in enumerate(((R, 0), (Zp, 128), (INN, 256))):
                        gs = slice(glo, glo + 128)
                        mms = []
                        for d in range(2):
                            reg = slice(d * b, (d + 1) * b)
                            for xt, wi in xkt:
                                mms.append((P[:, reg], wx_s[(l, d)][wi][:, gs], xt))
                            if t > 0 and gi_ < 2:
                                mms.append((P[:, reg], wh_s[(l, d)][:, gs],
                                            h_cur[l][:, reg]))
                        # identity MM closes the INN group (also at t==0 when
                        # a nonzero bhn requires the r*bhn correction)
                        open_group = (gi_ == 2 and (t > 0 or has_bias))
                        for i, (o_, w_, x_) in enumerate(mms):
                            nc.tensor.matmul(
                                out=o_, lhsT=r_(w_), rhs=r_(x_),
                                start=(i == 0),
                                stop=(i == len(mms) - 1) and not open_group)
                    if t > 0:
                        for d in range(2):
                            reg = slice(d * b, (d + 1) * b)
                            nc.tensor.matmul(
                                out=HN[:, reg], lhsT=r_(wh_s[(l, d)][:, 256:384]),
                                rhs=r_(h_cur[l][:, reg]), start=(d == 0),
                                stop=(d == 1))

                    rs = sp.tile([128, 2 * b], f32, tag="rs")
                    zs = sp.tile([128, 2 * b], f32, tag="zs")
                    ns = sp.tile([128, 2 * b], f32, tag="ns")
                    if has_bias:
                        for d in range(2):
                            reg = slice(d * b, (d + 1) * b)
                            nc.scalar.activation(rs[:, reg], R[:, reg], AF.Sigmoid,
                                                 bias=bget(f"br_{l}_{d}"))
                            nc.scalar.activation(zs[:, reg], Zp[:, reg], AF.Sigmoid,
                                                 bias=bget(f"bz_{l}_{d}"))
                    else:
                        nc.scalar.activation(rs, R, AF.Sigmoid)
                        nc.scalar.activation(zs, Zp, AF.Sigmoid)

                    if t > 0:
                        rhn = sp.tile([128, 2 * b], f32r, tag="rhn")
                        if has_bias:
                            for d in range(2):
                                reg = slice(d * b, (d + 1) * b)
                                nc.vector.scalar_tensor_tensor(
                                    out=rhn[:, reg], in0=HN[:, reg],
                                    scalar=bget(f"bhn_{l}_{d}"), in1=rs[:, reg],
                                    op0=OP.add, op1=OP.mult)
                        else:
                            nc.vector.scalar_tensor_tensor(
                                out=rhn, in0=HN, scalar=0.0, in1=rs,
                                op0=OP.add, op1=OP.mult)
                        nc.tensor.matmul(out=INN, lhsT=r_(ident), rhs=r_(rhn),
                                         start=False, stop=True)
                    elif has_bias:
                        # t == 0, h == 0: n = tanh(inn + bin + r*bhn)
                        rhn = sp.tile([128, 2 * b], f32r, tag="rhn", name="rhn0")
                        for d in range(2):
                            reg = slice(d * b, (d + 1) * b)
                            nc.vector.tensor_scalar(
                                out=rhn[:, reg], in0=rs[:, reg],
                                scalar1=bget(f"bhn_{l}_{d}"), scalar2=None,
                                op0=OP.mult)
                        nc.tensor.matmul(out=INN, lhsT=r_(ident), rhs=r_(rhn),
                                         start=False, stop=True)
                    if has_bias:
                        for d in range(2):
                            reg = slice(d * b, (d + 1) * b)
                            nc.scalar.activation(ns[:, reg], INN[:, reg], AF.Tanh,
                                                 bias=bget(f"bin_{l}_{d}"))
                    else:
                        nc.scalar.activation(ns, INN, AF.Tanh)

                    hnew = hp.tile([128, 2 * b], f32r, tag=f"h{l}")
                    u = sp.tile([128, 2 * b], f32, tag="u")
                    if t > 0:
                        dd = sp.tile([128, 2 * b], f32, tag="d")
                        nc.vector.tensor_tensor(dd, h_cur[l], ns, op=OP.subtract)
                        nc.vector.tensor_tensor(u, zs, dd, op=OP.mult)
                        nc.vector.tensor_tensor(hnew, ns, u, op=OP.add)
                    else:
                        nc.vector.tensor_tensor(u, zs, ns, op=OP.mult)
                        nc.vector.tensor_tensor(hnew, ns, u, op=OP.subtract)
                    h_cur[l] = hnew

                # ---- output head for step t ----------------------------
                htop = h_cur[3]
                feat_p = ps.tile([128, b], f32, tag="sm")
                nc.tensor.matmul(out=feat_p, lhsT=r_(woh_s), rhs=r_(htop[:, 0:b]),
                                 start=True, stop=False)
                nc.tensor.matmul(out=feat_p, lhsT=r_(woh_s), rhs=r_(htop[:, b:2 * b]),
                                 start=False, stop=False)
                nc.tensor.matmul(out=feat_p, lhsT=r_(woz_s), rhs=r_(zctx_sb),
                                 start=False, stop=True)
                feat_sb = sp.tile([128, b], f32r, tag="feat")
                if has_bias:
                    nc.scalar.activation(feat_sb, feat_p, AF.Identity,
                                         bias=bget("b_out"))
                else:
                    nc.scalar.copy(feat_sb, feat_p)

                b1_p = ps.tile([64, b], f32, tag="sm")
                nc.tensor.matmul(out=b1_p, lhsT=r_(wb1_s), rhs=r_(feat_sb),
                                 start=True, stop=True)
                b1_sb = sp.tile([64, b], f32r, tag="b1")
                if has_bias:
                    nc.scalar.activation(b1_sb, b1_p, AF.Identity, bias=bget("b_b1"))
                else:
                    nc.scalar.copy(b1_sb, b1_p)

                bt_p = ps.tile([27, b], f32, tag="sm")
                nc.tensor.matmul(out=bt_p, lhsT=r_(wb2_s), rhs=r_(b1_sb),
                                 start=True, stop=True)
                bt_sb = sp.tile([27, b], f32, tag="bt")
                if has_bias:
                    nc.scalar.activation(bt_sb, bt_p, AF.Identity, bias=bget("b_b2"))
                else:
                    nc.scalar.copy(bt_sb, bt_p)
                nc.sync.dma_start(out=beat_o[t], in_=bt_sb)

    nc.compile()
    return nc, wnames


# --------------------------------------------------------------------------
# host-side data prep
# --------------------------------------------------------------------------

def _prep_weights(inp, has_bias, t_steps=T):
    f = np.float32
    w = {}
    for d in range(2):
        wih0 = np.asarray(inp["w_ih_l0"][d], f)           # (384, 144)
        pose_w = np.asarray(inp["pose_w"], f)             # (16, 28)
        w[f"wxs0_{d}"] = np.ascontiguousarray(wih0[:, 16:144].T)       # (128,384)
        w[f"wxp0_{d}"] = np.ascontiguousarray((wih0[:, 0:16] @ pose_w).T)  # (28,384)
        w[f"whh0_{d}"] = np.ascontiguousarray(np.asarray(inp["w_hh_l0"][d], f).T)
    for l in range(1, 4):
        for d in range(2):
            wih = np.asarray(inp["w_ih"][l - 1][d], f).T  # (256, 384)
            w[f"wxa{l}_{d}"] = np.ascontiguousarray(wih[0:128])
            w[f"wxb{l}_{d}"] = np.ascontiguousarray(wih[128:256])
            w[f"whh{l}_{d}"] = np.ascontiguousarray(np.asarray(inp["w_hh"][l - 1][d], f).T)
    out_w = np.asarray(inp["out_w"], f)                   # (128, 144)
    w["woh"] = np.ascontiguousarray(out_w[:, 0:128].T)
    w["woz"] = np.ascontiguousarray(out_w[:, 128:144].T)
    w["wb1"] = np.ascontiguousarray(np.asarray(inp["bo_w1"], f).T)     # (128,64)
    w["wb2"] = np.ascontiguousarray(np.asarray(inp["bo_w2"], f).T)     # (64,27)
    w["wsqu"] = np.ascontiguousarray(np.asarray(inp["squ_w"], f).T)    # (90,128)
    w["woen"] = np.ascontiguousarray(np.asarray(inp["oenv_w"], f).T)
    w["wspk"] = np.ascontiguousarray(np.asarray(inp["spk_lin_w"], f).T)
    w["wmu"] = np.ascontiguousarray(np.asarray(inp["mu_w"], f).T)
    w["wlv"] = np.ascontiguousarray(np.asarray(inp["lv_w"], f).T)
    w["ident"] = np.eye(128, dtype=f)
    if has_bias:
        for l in range(4):
            bih = np.asarray(inp["b_ih_l0"] if l == 0 else inp["b_ih"][l - 1], f)
            bhh = np.asarray(inp["b_hh_l0"] if l == 0 else inp["b_hh"][l - 1], f)
            if l == 0:
                # pose MLP is folded into layer-0 weights; its bias flows
                # through the folded linear map into the x-side gate bias.
                pose_b = np.asarray(inp["pose_b"], f)
                bih = bih + np.stack(
                    [np.asarray(inp["w_ih_l0"][d], f)[:, 0:16] @ pose_b
                     for d in range(2)])
            for d in range(2):
                w[f"br_{l}_{d}"] = (bih[d, 0:128] + bhh[d, 0:128]).reshape(128, 1)
                w[f"bz_{l}_{d}"] = (bih[d, 128:256] + bhh[d, 128:256]).reshape(128, 1)
                w[f"bin_{l}_{d}"] = bih[d, 256:384].reshape(128, 1).astype(f)
                w[f"bhn_{l}_{d}"] = bhh[d, 256:384].reshape(128, 1).astype(f)
        w["b_out"] = np.asarray(inp["out_b"], f).reshape(128, 1)
        w["b_b1"] = np.asarray(inp["bo_b1"], f).reshape(64, 1)
        w["b_b2"] = np.asarray(inp["bo_b2"], f).reshape(27, 1)
        w["b_sum"] = (np.asarray(inp["squ_b"], f)
                      + np.asarray(inp["oenv_b"], f)).reshape(128, 1)
        w["b_spk"] = np.asarray(inp["spk_lin_b"], f).reshape(16, 1)
        w["b_mu"] = np.asarray(inp["mu_b"], f).reshape(16, 1)
        w["b_lv"] = np.asarray(inp["lv_b"], f).reshape(16, 1)
        w["b_lvh"] = (0.5 * np.asarray(inp["lv_b"], f)).reshape(16, 1)
    return w


def _has_bias(inp):
    keys = ["b_ih_l0", "b_hh_l0", "b_ih", "b_hh", "squ_b", "oenv_b", "pose_b",
            "out_b", "bo_b1", "bo_b2", "spk_lin_b", "mu_b", "lv_b"]
    return any(np.any(np.asarray(inp[k]) != 0) for k in keys)


def _shard_inputs(inp, core, w, t_steps=T, b=B):
    f = np.float32
    sl = slice(core * b, (core + 1) * b)
    beats = np.asarray(inp["beats"][sl], f)               # (b, 2, T*SIDE)
    bt = beats.transpose(1, 2, 0)                         # (2, T*SIDE, b)
    pad = np.zeros((2, SIDE, b), f)
    beats_t = np.ascontiguousarray(np.concatenate([pad, bt, pad], axis=1))
    pre = np.asarray(inp["pre_seq"][sl], f)               # (b, T, 28)
    pre_t = np.ascontiguousarray(pre.transpose(1, 2, 0))  # (T, 28, b)
    vid = np.asarray(inp["vid_indices"][sl])
    spk_t = np.ascontiguousarray(np.asarray(inp["spk_table"], f)[vid].T)  # (16,b)
    eps_t = np.ascontiguousarray(np.asarray(inp["eps"][sl], f).T)         # (16,b)
    m = {"beats_t": beats_t, "pre_t": pre_t, "spk_t": spk_t, "eps_t": eps_t}
    m.update(w)
    return m


def timeline_estimate(trace_path=None, has_bias=False):
    """Cost-model estimate (ns) of single-core exec time; optional perfetto."""
    key = ("prog", has_bias)
    if key not in _CACHE:
        _CACHE[key] = _build(has_bias)
    nc, _ = _CACHE[key]
    from concourse.timeline_sim import TimelineSim
    ts = TimelineSim(nc, trace=trace_path is not None)
    t = ts.simulate()
    if trace_path is not None and ts.perfetto is not None:
        ts.perfetto.save(trace_path)
    return int(t)


def kernel(**inputs):
    inputs = {k: np.asarray(v) for k, v in inputs.items()}
    has_bias = _has_bias(inputs)
    key = ("prog", has_bias)
    if key not in _CACHE:
        _CACHE[key] = _build(has_bias)
    nc, _ = _CACHE[key]

    w = _prep_weights(inputs, has_bias)
    in_maps = [_shard_inputs(inputs, c, w) for c in range(NCORES)]

    from concourse.bass_utils import run_bass_kernel_spmd
    res = run_bass_kernel_spmd(nc, in_maps, core_ids=list(range(NCORES)))
    _CACHE["last_res"] = res

    beat = np.zeros((BS, T, 27), np.float32)
    zc = np.zeros((BS, Z), np.float32)
    zmu = np.zeros((BS, Z), np.float32)
    zlv = np.zeros((BS, Z), np.float32)
    for c, r in enumerate(res.results):
        sl = slice(c * B, (c + 1) * B)
        beat[sl] = r["beat_o"].transpose(2, 0, 1)         # (T,27,b) -> (b,T,27)
        zc[sl] = r["zc_o"].T
        zmu[sl] = r["zmu_o"].T
        zlv[sl] = r["zlv_o"].T
    return beat, zc, zmu, zlv
